# revision 2
# baseline (speedup 1.0000x reference)
"""Trainium2 Bass kernel for nn_DecoderStory_71880572666639 (V2).

Architecture: 2-layer LSTM (H=512) scanned sequentially over the compacted
valid (b,t) steps (hidden carries across the whole batch), followed by a
vocab projection V=10000.

V2 strategy (vs V1 baseline):
* Two independent sequential chains: layer-1 (needs only X1[t] + h1 rec)
  and layer-2 (needs X2[t] = W_ih2 @ h1[t] + b2 + h2 rec). Layer-2 runs C=U
  steps behind layer-1 on the SAME core; X2 is computed in chunked GEMMs
  (amortizing weight loads), not per-step matvecs. Per-step PE work drops
  from 192 to 128+eps weight tiles.
* Recurrent weights + hidden states in fp8e4 (x64 / x16 scale folding,
  un-scaled in the gate activation via ACT's scale arg) -> faster LDWEIGHTS.
* Gate tiles reordered (i,f,o | g~) so gate nonlinearity is 2 fused ACT
  instructions ([128,12] sigmoid + [128,4] tanh) instead of 32 per-column.
* The precomputed input projection X[t] is seeded into PSUM with a single
  identity matmul (start=True), so no per-column ACT bias is needed.
* All 8 cores run the identical scan; the vocab GEMM is split by vocab
  columns (1250 per core) exactly like V1.

Gate permutation: device gate j = 128*m + p (column m in [0,16), partition
p) maps to torch gate g = 512*T[m//4] + 128*(m%4) + p with T = (i,f,o,g~) =
(0,1,3,2); so columns 0-3 hold i, 4-7 f, 8-11 o, 12-15 g~, and hidden unit
u = 128*c + p lives at h[p, c] for c in [0,4).
"""

import os
import numpy as np

B, T, E, H, V = 64, 32, 256, 512, 10000
D1 = E + H            # 768
G = 4 * H             # 2048
P = 128
NCORES = 8
VSLICE = V // NCORES  # 1250
KC1 = D1 // P         # 6  K-chunks for the input projection
KH = H // P           # 4  K-chunks for one hidden vector
MG = G // P           # 16 gate tiles

U = int(os.environ.get("SCAN_UNROLL", "8"))     # unroll = chainB lag C
WDT_NAME = os.environ.get("SCAN_WDT", "fp8")    # "fp8" | "fp16"
WS = 64.0             # recurrent weight scale
HS = 16.0             # hidden state scale
XS = WS * HS          # 1024: scale of X1/X2/psum gate pre-activations
FLUSH = -60000.0      # forces sigmoid gates to 0 in chainB warmup steps

# device gate permutation (device j -> torch gate index); type order i,f,o,g
_m = np.arange(G) // P
_p = np.arange(G) % P
_TY = np.array([0, 1, 3, 2])  # i, f, o, g~
PERM = 512 * _TY[_m // 4] + P * (_m % 4) + _p      # [2048]


def _round_up(x, mult):
    return ((x + mult - 1) // mult) * mult


# ---------------------------------------------------------------------------
# host-side packing
# ---------------------------------------------------------------------------

def _pack_stationary(Wp: np.ndarray, kchunks: int) -> np.ndarray:
    """Pack a permuted weight matrix Wp [G, K] into the SBUF stationary
    layout [128, (MG*kchunks)*128], block order b = m*kchunks + k,
    block(m, k)[kk, mm] = Wp[128*m + mm, 128*k + kk]."""
    ksz = Wp.shape[1]
    assert ksz == kchunks * P
    v = Wp.reshape(MG, P, kchunks, P)           # [m, mm, k, kk]
    v = v.transpose(3, 0, 2, 1)                 # [kk, m, k, mm]
    return np.ascontiguousarray(v.reshape(P, MG * kchunks * P))


def _host_pack(story_feature, captions, lengths, W_story, b_story, embed,
               W_ih1, W_hh1, b1, W_ih2, W_hh2, b2, W_out, b_out):
    f32 = np.float32
    f16 = np.float16
    feats = np.maximum(story_feature.astype(f32) @ W_story.T.astype(f32)
                       + b_story.astype(f32), 0.0)          # [B, H]

    lengths = lengths.astype(np.int64)
    valid_pairs = [(b, t) for b in range(B) for t in range(int(lengths[b]) - 1)]
    n_valid = len(valid_pairs)
    L = max(_round_up(n_valid, max(P, U)), 2 * P)

    # x rows [feats; emb] in fp16, zero-padded to L+U steps
    x = np.zeros((L + U, D1), f16)
    bs = np.array([p[0] for p in valid_pairs])
    ts = np.array([p[1] for p in valid_pairs])
    x[:n_valid, :H] = feats[bs].astype(f16)
    x[:n_valid, H:] = embed[captions[bs, ts]].astype(f16)

    # xts: x.T chunked [128, KC1 * (L+U)]
    xT = np.ascontiguousarray(x.T)                        # [768, L+U]
    xts = xT.reshape(KC1, P, L + U).transpose(1, 0, 2).reshape(P, KC1 * (L + U))

    w1i = _pack_stationary(W_ih1[PERM].astype(f32) * XS, KC1).astype(f16)
    b1s = (b1[PERM].astype(f32) * XS).astype(f16).reshape(1, G)

    wa = _pack_stationary(W_hh1[PERM].astype(f32) * WS, KH)
    wb = _pack_stationary(W_hh2[PERM].astype(f32) * WS, KH)
    wi2 = _pack_stationary(W_ih2[PERM].astype(f32) * WS, KH)

    # b2 replicated over the GEMM window, layout [128, 16, U]
    b2t = np.ascontiguousarray(b2[PERM].reshape(MG, P).T).astype(f32) * XS
    b2rep = np.repeat(b2t[:, :, None], U, axis=2).astype(f16)  # [128,16,U]

    ident = np.eye(P, dtype=f16)

    # per-core W_out slices: woutt[kk, c*VSLICE + v] = W_out[v0+v, 128c+kk]
    wouts = []
    for core in range(NCORES):
        Woc = W_out[core * VSLICE:(core + 1) * VSLICE].astype(f32)   # [1250, 512]
        wt = Woc.T.reshape(KH, P, VSLICE).transpose(1, 0, 2).reshape(P, KH * VSLICE)
        wouts.append(np.ascontiguousarray(wt).astype(f16))

    meta = dict(n_valid=n_valid, L=L, bs=bs, ts=ts)
    dev = dict(
        xts=np.ascontiguousarray(xts).astype(f16),
        w1i=w1i, b1s=b1s, wa=wa, wb=wb, wi2=wi2,
        b2rep=np.ascontiguousarray(b2rep.reshape(P, MG * U)),
        ident=ident, wouts=wouts,
    )
    return dev, meta


# ---------------------------------------------------------------------------
# device kernel build
# ---------------------------------------------------------------------------

_BUILD_CACHE = {}


def _build(L):
    import concourse.bass as bass
    import concourse.tile as tile
    from concourse import bacc, mybir
    from concourse.bass import ds
    from contextlib import ExitStack

    F32 = mybir.dt.float32
    F16 = mybir.dt.float16
    WDT = mybir.dt.float8e4 if WDT_NAME == "fp8" else mybir.dt.float16
    AF = mybir.ActivationFunctionType
    NITER = L // U + 1
    SLOTS = L + U + 1          # h-state slots per chain

    nc = bacc.Bacc("TRN2", target_bir_lowering=False, debug=False,
                   num_devices=NCORES)

    xts_d = nc.dram_tensor("xts", [P, KC1 * (L + U)], F16, kind="ExternalInput").ap()
    w1i_d = nc.dram_tensor("w1i", [P, MG * KC1 * P], F16, kind="ExternalInput").ap()
    b1s_d = nc.dram_tensor("b1s", [1, G], F16, kind="ExternalInput").ap()
    wa_d = nc.dram_tensor("wa", [P, MG * KH * P], WDT, kind="ExternalInput").ap()
    wb_d = nc.dram_tensor("wb", [P, MG * KH * P], WDT, kind="ExternalInput").ap()
    wi2_d = nc.dram_tensor("wi2", [P, MG * KH * P], WDT, kind="ExternalInput").ap()
    b2rep_d = nc.dram_tensor("b2rep", [P, MG * U], F16, kind="ExternalInput").ap()
    ident_d = nc.dram_tensor("ident", [P, P], F16, kind="ExternalInput").ap()
    wout_d = nc.dram_tensor("woutt", [P, KH * VSLICE], F16, kind="ExternalInput").ap()
    out_d = nc.dram_tensor("out", [L, VSLICE], F32, kind="ExternalOutput").ap()

    with tile.TileContext(nc) as tc:
        with ExitStack() as ctx:
            singles = ctx.enter_context(tc.tile_pool(name="singles", bufs=1))
            stage = ctx.enter_context(tc.tile_pool(name="stage", bufs=2))

            # --- persistent SBUF tensors ---
            wa = singles.tile([P, MG * KH * P], WDT)
            wb = singles.tile([P, MG * KH * P], WDT)
            wi2 = singles.tile([P, MG * KH * P], WDT)
            w1i = singles.tile([P, MG * KC1 * P], F16)
            b1s = singles.tile([1, G], F16)
            b2rep = singles.tile([P, MG, U], F16)
            ident = singles.tile([P, P], F16)
            woutt = singles.tile([P, KH * VSLICE], F16)
            ones = singles.tile([1, 512], F16)
            nc.sync.dma_start(out=wa, in_=wa_d)
            nc.sync.dma_start(out=wb, in_=wb_d)
            nc.sync.dma_start(out=wi2, in_=wi2_d)
            nc.sync.dma_start(out=w1i, in_=w1i_d)
            nc.sync.dma_start(out=b1s, in_=b1s_d)
            nc.sync.dma_start(out=b2rep, in_=b2rep_d.rearrange("p (m u) -> p m u", u=U))
            nc.sync.dma_start(out=ident, in_=ident_d)
            nc.sync.dma_start(out=woutt, in_=wout_d)
            nc.vector.memset(ones, 1.0)

            X1T = singles.tile([P, 16 * (L + U)], F16)
            X2T = singles.tile([P, 16 * (L + 2 * U)], F16)
            H1 = singles.tile([P, 4 * SLOTS], WDT)
            H2 = singles.tile([P, 4 * SLOTS], WDT)
            YS = singles.tile([P, 4 * (L + U)], F16)

            # zero init / flush regions
            nc.vector.memset(H1[:, 0:4], 0.0)
            nc.vector.memset(H2[:, 0:4], 0.0)
            nc.vector.memset(X1T[:, 16 * L:], 0.0)
            x2v = X2T.rearrange("p (s m) -> p s m", m=16)
            nc.vector.memset(x2v[:, 0:U, 0:12], FLUSH)
            nc.vector.memset(x2v[:, 0:U, 12:16], 0.0)

            c1 = singles.tile([P, KH], F32)
            c2 = singles.tile([P, KH], F32)
            nc.vector.memset(c1, 0.0)
            nc.vector.memset(c2, 0.0)

            # --- phase B: input projection X1 = XS*(W_ih1 @ x + b1) ---
            xts_v = xts_d.rearrange("p (k l) -> p k l", k=KC1)
            x1tv = X1T.rearrange("p (s m) -> p m s", m=16)
            nts = [(o, min(512, L - o)) for o in range(0, L, 512)]
            with tc.tile_pool(name="xpool", bufs=2) as xpool, \
                 tc.tile_pool(name="pre_ps", bufs=4, space="PSUM") as pre_ps:
                for (off, nlen) in nts:
                    xsl = xpool.tile([P, KC1, 512], F16, tag="xsl")
                    nc.sync.dma_start(out=xsl[:, :, :nlen], in_=xts_v[:, :, off:off + nlen])
                    for m in range(MG):
                        ps = pre_ps.tile([P, 512], F32, tag="ps")
                        nc.tensor.matmul(ps[:, :nlen], b1s[0:1, P * m:P * (m + 1)],
                                         ones[0:1, :nlen], start=True, stop=False)
                        for kc in range(KC1):
                            blk = w1i[:, (m * KC1 + kc) * P:(m * KC1 + kc + 1) * P]
                            nc.tensor.matmul(ps[:, :nlen], blk, xsl[:, kc, :nlen],
                                             start=False, stop=(kc == KC1 - 1))
                        nc.vector.tensor_copy(x1tv[:, m, off:off + nlen], ps[:, :nlen])

            # --- the scan ---
            h1v = H1.rearrange("p (s c) -> p c s", c=KH)
            x2wv = X2T.rearrange("p (s m) -> p m s", m=16)

            def chain_step(scan_ps, W, XT, HB, cS, e, tag, ys=False):
                g = scan_ps.tile([P, MG], F32, tag=f"g{tag}")
                sg = stage.tile([P, MG], F16, tag=f"sg{tag}")
                th = stage.tile([P, KH], F16, tag=f"th{tag}")
                t1 = stage.tile([P, KH], F32, tag=f"t1{tag}")
                t2 = stage.tile([P, KH], F32, tag=f"t2{tag}")
                # gates = XS*X[t] (seed) + (WS*W) @ (HS*h)
                nc.tensor.matmul(g[:, 0:MG], ident, XT[:, ds(e * MG, MG)],
                                 start=True, stop=False)
                for m in range(MG):
                    for c in range(KH):
                        blk = W[:, (m * KH + c) * P:(m * KH + c + 1) * P]
                        nc.tensor.matmul(g[:, m:m + 1], blk, HB[:, ds(e * KH + c, 1)],
                                         start=False,
                                         stop=(m == MG - 1 and c == KH - 1))
                nc.scalar.activation(sg[:, 0:12], g[:, 0:12], AF.Sigmoid,
                                     scale=1.0 / XS)
                nc.scalar.activation(sg[:, 12:16], g[:, 12:16], AF.Tanh,
                                     scale=1.0 / XS)
                # c = f*c + i*g~ ; h = o*tanh(c)
                nc.vector.tensor_mul(t2, sg[:, 4:8], cS)
                nc.vector.tensor_mul(t1, sg[:, 0:4], sg[:, 12:16])
                nc.vector.tensor_add(cS, t2, t1)
                nc.scalar.activation(th, cS, AF.Tanh)
                # fp8 record (x HS) consumed by the recurrent matvec
                nc.vector.scalar_tensor_tensor(
                    HB[:, ds(e * KH + KH, KH)], sg[:, 8:12], HS, th,
                    op0=mybir.AluOpType.mult, op1=mybir.AluOpType.mult)
                if ys:
                    nc.vector.tensor_mul(YS[:, ds(e * KH, KH)], sg[:, 8:12], th)

            with tc.tile_pool(name="scan_ps", bufs=2, space="PSUM") as scan_ps, \
                 tc.tile_pool(name="x2_ps", bufs=2, space="PSUM") as x2_ps:
                with tc.For_i(0, NITER, 1,
                              hint_engines=(mybir.EngineType.PE,)) as i:
                    for u in range(U):
                        e = nc.snap(i * U + u)
                        chain_step(scan_ps, wa, X1T, H1, c1, e, "a")
                        chain_step(scan_ps, wb, X2T, H2, c2, e, "b", ys=True)
                    # X2 GEMM for next iteration's chain-B window
                    x2p = x2_ps.tile([P, MG, U], F32, tag="x2p")
                    for m in range(MG):
                        for c in range(KH):
                            blk = wi2[:, (m * KH + c) * P:(m * KH + c + 1) * P]
                            nc.tensor.matmul(x2p[:, m, :], blk,
                                             h1v[:, c, ds(i * U + 1, U)],
                                             start=(c == 0), stop=(c == KH - 1))
                    nc.vector.tensor_add(x2wv[:, :, ds(i * U + U, U)], x2p, b2rep)

            # --- phase D: vocab projection (ys rows start at slot U) ---
            ys_v = YS.rearrange("p (s c) -> p c s", c=KH)
            vts = [(o, min(512, VSLICE - o)) for o in range(0, VSLICE, 512)]
            gemm_ps = ctx.enter_context(tc.tile_pool(name="gemm_ps", bufs=2, space="PSUM"))
            for sb in range(L // P):
                for (voff, vlen) in vts:
                    ps = gemm_ps.tile([P, 512], F32, tag="gps")
                    for c in range(KH):
                        nc.tensor.matmul(ps[:, :vlen],
                                         ys_v[:, c, U + P * sb:U + P * (sb + 1)],
                                         woutt[:, c * VSLICE + voff:c * VSLICE + voff + vlen],
                                         start=(c == 0), stop=(c == KH - 1))
                    st = stage.tile([P, 512], F32, tag="gst")
                    nc.scalar.copy(st[:, :vlen], ps[:, :vlen])
                    nc.sync.dma_start(out=out_d[P * sb:P * (sb + 1), voff:voff + vlen],
                                      in_=st[:, :vlen])

    nc.compile()
    return nc


# ---------------------------------------------------------------------------
# public entry point
# ---------------------------------------------------------------------------

LAST_RESULT = None


def _in_maps(dev):
    maps = []
    for core in range(NCORES):
        maps.append(dict(
            xts=dev["xts"], w1i=dev["w1i"], b1s=dev["b1s"], wa=dev["wa"],
            wb=dev["wb"], wi2=dev["wi2"], b2rep=dev["b2rep"],
            ident=dev["ident"], woutt=dev["wouts"][core],
        ))
    return maps


def _cast_wdt(dev):
    """Cast the packed recurrent weights to the device dtype (ml_dtypes for
    fp8)."""
    if WDT_NAME == "fp8":
        import ml_dtypes
        dt = ml_dtypes.float8_e4m3
    else:
        dt = np.float16
    for k in ("wa", "wb", "wi2"):
        dev[k] = dev[k].astype(dt)
    return dev


def kernel(story_feature, captions, lengths, W_story, b_story, embed,
           W_ih1, W_hh1, b1, W_ih2, W_hh2, b2, W_out, b_out):
    global LAST_RESULT
    from concourse import bass_utils

    dev, meta = _host_pack(story_feature, captions, lengths, W_story, b_story,
                           embed, W_ih1, W_hh1, b1, W_ih2, W_hh2, b2, W_out, b_out)
    dev = _cast_wdt(dev)
    L = meta["L"]

    key = (L, U, WDT_NAME)
    if key not in _BUILD_CACHE:
        _BUILD_CACHE[key] = _build(L)
    nc = _BUILD_CACHE[key]

    trace = os.environ.get("BASS_TRACE", "0") == "1"
    res = bass_utils.run_bass_kernel_spmd(nc, _in_maps(dev),
                                          core_ids=list(range(NCORES)),
                                          trace=trace)
    LAST_RESULT = res

    logits = np.concatenate([res.results[c]["out"] for c in range(NCORES)],
                            axis=1)            # [L, V]
    return _host_post(logits, meta, b_out)


def _host_post(logits, meta, b_out):
    n_valid, bs, ts = meta["n_valid"], meta["bs"], meta["ts"]
    out = np.zeros((B, T, V), np.float32)
    out[:, 0, 1] = 10000.0
    rows = logits[:n_valid].astype(np.float32) + b_out.astype(np.float32)[None, :]
    out[bs, ts + 1] = rows
    return out


# revision 3
# speedup vs baseline: 1.0240x; 1.0240x over previous
"""Trainium2 Bass kernel for nn_DecoderStory_71880572666639 (V2).

Architecture: 2-layer LSTM (H=512) scanned sequentially over the compacted
valid (b,t) steps (hidden carries across the whole batch), followed by a
vocab projection V=10000.

V2 strategy (vs V1 baseline):
* Two independent sequential chains: layer-1 (needs only X1[t] + h1 rec)
  and layer-2 (needs X2[t] = W_ih2 @ h1[t] + b2 + h2 rec). Layer-2 runs C=U
  steps behind layer-1 on the SAME core; X2 is computed in chunked GEMMs
  (amortizing weight loads), not per-step matvecs. Per-step PE work drops
  from 192 to 128+eps weight tiles.
* Recurrent weights + hidden states in fp8e4 (x64 / x16 scale folding,
  un-scaled in the gate activation via ACT's scale arg) -> faster LDWEIGHTS.
* Gate tiles reordered (i,f,o | g~) so gate nonlinearity is 2 fused ACT
  instructions ([128,12] sigmoid + [128,4] tanh) instead of 32 per-column.
* The precomputed input projection X[t] is seeded into PSUM with a single
  identity matmul (start=True), so no per-column ACT bias is needed.
* All 8 cores run the identical scan; the vocab GEMM is split by vocab
  columns (1250 per core) exactly like V1.

Gate permutation: device gate j = 128*m + p (column m in [0,16), partition
p) maps to torch gate g = 512*T[m//4] + 128*(m%4) + p with T = (i,f,o,g~) =
(0,1,3,2); so columns 0-3 hold i, 4-7 f, 8-11 o, 12-15 g~, and hidden unit
u = 128*c + p lives at h[p, c] for c in [0,4).
"""

import os
import numpy as np

B, T, E, H, V = 64, 32, 256, 512, 10000
D1 = E + H            # 768
G = 4 * H             # 2048
P = 128
NCORES = 8
VSLICE = V // NCORES  # 1250
KC1 = D1 // P         # 6  K-chunks for the input projection
KH = H // P           # 4  K-chunks for one hidden vector
MG = G // P           # 16 gate tiles

U = int(os.environ.get("SCAN_UNROLL", "8"))     # unroll = chainB lag C
WDT_NAME = os.environ.get("SCAN_WDT", "fp8")    # "fp8" | "fp16"
WS = 64.0             # recurrent weight scale
HS = 16.0             # hidden state scale
XS = WS * HS          # 1024: scale of X1/X2/psum gate pre-activations
FLUSH = -60000.0      # forces sigmoid gates to 0 in chainB warmup steps

# device gate permutation (device j -> torch gate index); type order i,f,o,g
_m = np.arange(G) // P
_p = np.arange(G) % P
_TY = np.array([0, 1, 3, 2])  # i, f, o, g~
PERM = 512 * _TY[_m // 4] + P * (_m % 4) + _p      # [2048]


def _round_up(x, mult):
    return ((x + mult - 1) // mult) * mult


# ---------------------------------------------------------------------------
# host-side packing
# ---------------------------------------------------------------------------

def _pack_stationary(Wp: np.ndarray, kchunks: int) -> np.ndarray:
    """Pack a permuted weight matrix Wp [G, K] into the SBUF stationary
    layout [128, (MG*kchunks)*128], block order b = m*kchunks + k,
    block(m, k)[kk, mm] = Wp[128*m + mm, 128*k + kk]."""
    ksz = Wp.shape[1]
    assert ksz == kchunks * P
    v = Wp.reshape(MG, P, kchunks, P)           # [m, mm, k, kk]
    v = v.transpose(3, 0, 2, 1)                 # [kk, m, k, mm]
    return np.ascontiguousarray(v.reshape(P, MG * kchunks * P))


def _host_pack(story_feature, captions, lengths, W_story, b_story, embed,
               W_ih1, W_hh1, b1, W_ih2, W_hh2, b2, W_out, b_out):
    f32 = np.float32
    f16 = np.float16
    feats = np.maximum(story_feature.astype(f32) @ W_story.T.astype(f32)
                       + b_story.astype(f32), 0.0)          # [B, H]

    # tanh is computed as 2*sigmoid(2x)-1 on device (single ACT table set);
    # fold the 2x into the g~ gate rows (torch rows [1024,1536)).
    gsc = np.ones((G, 1), np.float32)
    gsc[1024:1536] = 2.0
    W_ih1 = W_ih1 * gsc; W_hh1 = W_hh1 * gsc; b1 = b1 * gsc[:, 0]
    W_ih2 = W_ih2 * gsc; W_hh2 = W_hh2 * gsc; b2 = b2 * gsc[:, 0]

    lengths = lengths.astype(np.int64)
    valid_pairs = [(b, t) for b in range(B) for t in range(int(lengths[b]) - 1)]
    n_valid = len(valid_pairs)
    L = max(_round_up(n_valid, max(P, U)), 2 * P)

    # x rows [feats; emb] in fp16, zero-padded to L+U steps
    x = np.zeros((L + U, D1), f16)
    bs = np.array([p[0] for p in valid_pairs])
    ts = np.array([p[1] for p in valid_pairs])
    x[:n_valid, :H] = feats[bs].astype(f16)
    x[:n_valid, H:] = embed[captions[bs, ts]].astype(f16)

    # xts: x.T chunked [128, KC1 * (L+U)]
    xT = np.ascontiguousarray(x.T)                        # [768, L+U]
    xts = xT.reshape(KC1, P, L + U).transpose(1, 0, 2).reshape(P, KC1 * (L + U))

    w1i = _pack_stationary(W_ih1[PERM].astype(f32) * XS, KC1).astype(f16)
    b1s = (b1[PERM].astype(f32) * XS).astype(f16).reshape(1, G)

    wa = _pack_stationary(W_hh1[PERM].astype(f32) * WS, KH)
    wb = _pack_stationary(W_hh2[PERM].astype(f32) * WS, KH)
    wi2 = _pack_stationary(W_ih2[PERM].astype(f32) * WS, KH)

    # b2 replicated over the GEMM window, layout [128, 16, U]
    b2t = np.ascontiguousarray(b2[PERM].reshape(MG, P).T).astype(f32) * XS
    b2rep = np.repeat(b2t[:, :, None], U, axis=2).astype(f16)  # [128,16,U]

    ident = np.eye(P, dtype=f16)

    # per-core W_out slices: woutt[kk, c*VSLICE + v] = W_out[v0+v, 128c+kk]
    wouts = []
    for core in range(NCORES):
        Woc = W_out[core * VSLICE:(core + 1) * VSLICE].astype(f32)   # [1250, 512]
        wt = Woc.T.reshape(KH, P, VSLICE).transpose(1, 0, 2).reshape(P, KH * VSLICE)
        wouts.append(np.ascontiguousarray(wt).astype(f16))

    meta = dict(n_valid=n_valid, L=L, bs=bs, ts=ts)
    dev = dict(
        xts=np.ascontiguousarray(xts).astype(f16),
        w1i=w1i, b1s=b1s, wa=wa, wb=wb, wi2=wi2,
        b2rep=np.ascontiguousarray(b2rep.reshape(P, MG * U)),
        ident=ident, wouts=wouts,
    )
    return dev, meta


# ---------------------------------------------------------------------------
# device kernel build
# ---------------------------------------------------------------------------

_BUILD_CACHE = {}


def _build(L):
    import concourse.bass as bass
    import concourse.tile as tile
    from concourse import bacc, mybir
    from concourse.bass import ds
    from contextlib import ExitStack

    F32 = mybir.dt.float32
    F16 = mybir.dt.float16
    WDT = mybir.dt.float8e4 if WDT_NAME == "fp8" else mybir.dt.float16
    AF = mybir.ActivationFunctionType
    NITER = L // U + 1
    SLOTS = L + U + 1          # h-state slots per chain

    nc = bacc.Bacc("TRN2", target_bir_lowering=False, debug=False,
                   num_devices=NCORES)

    xts_d = nc.dram_tensor("xts", [P, KC1 * (L + U)], F16, kind="ExternalInput").ap()
    w1i_d = nc.dram_tensor("w1i", [P, MG * KC1 * P], F16, kind="ExternalInput").ap()
    b1s_d = nc.dram_tensor("b1s", [1, G], F16, kind="ExternalInput").ap()
    wa_d = nc.dram_tensor("wa", [P, MG * KH * P], WDT, kind="ExternalInput").ap()
    wb_d = nc.dram_tensor("wb", [P, MG * KH * P], WDT, kind="ExternalInput").ap()
    wi2_d = nc.dram_tensor("wi2", [P, MG * KH * P], WDT, kind="ExternalInput").ap()
    b2rep_d = nc.dram_tensor("b2rep", [P, MG * U], F16, kind="ExternalInput").ap()
    ident_d = nc.dram_tensor("ident", [P, P], F16, kind="ExternalInput").ap()
    wout_d = nc.dram_tensor("woutt", [P, KH * VSLICE], F16, kind="ExternalInput").ap()
    out_d = nc.dram_tensor("out", [L, VSLICE], F32, kind="ExternalOutput").ap()

    with tile.TileContext(nc) as tc:
        with ExitStack() as ctx:
            singles = ctx.enter_context(tc.tile_pool(name="singles", bufs=1))
            stage = ctx.enter_context(tc.tile_pool(name="stage", bufs=2))

            # --- persistent SBUF tensors ---
            wa = singles.tile([P, MG * KH * P], WDT)
            wb = singles.tile([P, MG * KH * P], WDT)
            wi2 = singles.tile([P, MG * KH * P], WDT)
            w1i = singles.tile([P, MG * KC1 * P], F16)
            b1s = singles.tile([1, G], F16)
            b2rep = singles.tile([P, MG, U], F16)
            ident = singles.tile([P, P], F16)
            woutt = singles.tile([P, KH * VSLICE], F16)
            ones = singles.tile([1, 512], F16)
            nc.sync.dma_start(out=wa, in_=wa_d)
            nc.sync.dma_start(out=wb, in_=wb_d)
            nc.sync.dma_start(out=wi2, in_=wi2_d)
            nc.sync.dma_start(out=w1i, in_=w1i_d)
            nc.sync.dma_start(out=b1s, in_=b1s_d)
            nc.sync.dma_start(out=b2rep, in_=b2rep_d.rearrange("p (m u) -> p m u", u=U))
            nc.sync.dma_start(out=ident, in_=ident_d)
            nc.sync.dma_start(out=woutt, in_=wout_d)
            nc.vector.memset(ones, 1.0)

            X1T = singles.tile([P, 16 * (L + U)], F16)
            X2T = singles.tile([P, 16 * (L + 2 * U)], F16)
            H1 = singles.tile([P, 4 * SLOTS], WDT)
            H2 = singles.tile([P, 4 * SLOTS], WDT)
            YS = singles.tile([P, 4 * (L + U)], F16)

            # zero init / flush regions
            nc.vector.memset(H1[:, 0:4], 0.0)
            nc.vector.memset(H2[:, 0:4], 0.0)
            nc.vector.memset(X1T[:, 16 * L:], 0.0)
            x2v = X2T.rearrange("p (s m) -> p s m", m=16)
            nc.vector.memset(x2v[:, 0:U, 0:12], FLUSH)
            nc.vector.memset(x2v[:, 0:U, 12:16], 0.0)

            c1 = singles.tile([P, KH], F32)
            c2 = singles.tile([P, KH], F32)
            nc.vector.memset(c1, 0.0)
            nc.vector.memset(c2, 0.0)

            # --- phase B: input projection X1 = XS*(W_ih1 @ x + b1) ---
            xts_v = xts_d.rearrange("p (k l) -> p k l", k=KC1)
            x1tv = X1T.rearrange("p (s m) -> p m s", m=16)
            nts = [(o, min(512, L - o)) for o in range(0, L, 512)]
            with tc.tile_pool(name="xpool", bufs=2) as xpool, \
                 tc.tile_pool(name="pre_ps", bufs=4, space="PSUM") as pre_ps:
                for (off, nlen) in nts:
                    xsl = xpool.tile([P, KC1, 512], F16, tag="xsl")
                    nc.sync.dma_start(out=xsl[:, :, :nlen], in_=xts_v[:, :, off:off + nlen])
                    for m in range(MG):
                        ps = pre_ps.tile([P, 512], F32, tag="ps")
                        nc.tensor.matmul(ps[:, :nlen], b1s[0:1, P * m:P * (m + 1)],
                                         ones[0:1, :nlen], start=True, stop=False)
                        for kc in range(KC1):
                            blk = w1i[:, (m * KC1 + kc) * P:(m * KC1 + kc + 1) * P]
                            nc.tensor.matmul(ps[:, :nlen], blk, xsl[:, kc, :nlen],
                                             start=False, stop=(kc == KC1 - 1))
                        nc.vector.tensor_copy(x1tv[:, m, off:off + nlen], ps[:, :nlen])

            # --- the scan ---
            h1v = H1.rearrange("p (s c) -> p c s", c=KH)
            x2wv = X2T.rearrange("p (s m) -> p m s", m=16)

            ones4 = singles.tile([P, KH], F16)
            nc.vector.memset(ones4, 1.0)

            def chain_step(scan_ps, W, XT, HB, cS, e, tag, ys=False):
                # gate cols: i=0:4 f=4:8 o=8:12 sg~=12:16 (all sigmoid;
                # tanh(x) == 2*sigmoid(2x)-1 with the 2x pre-folded)
                g = scan_ps.tile([P, MG], F32, tag=f"g{tag}")
                sg = stage.tile([P, MG], F16, tag=f"sg{tag}")
                sc = stage.tile([P, KH], F16, tag=f"sc{tag}")
                w4 = stage.tile([P, KH], F16, tag=f"w4{tag}")
                t1 = stage.tile([P, KH], F32, tag=f"t1{tag}")
                t2 = stage.tile([P, KH], F32, tag=f"t2{tag}")
                tm = stage.tile([P, KH], F32, tag=f"tm{tag}")
                # gates = XS*X[t] (seed) + (WS*W) @ (HS*h)
                nc.tensor.matmul(g[:, 0:MG], ident, XT[:, ds(e * MG, MG)],
                                 start=True, stop=False)
                for m in range(MG):
                    for c in range(KH):
                        blk = W[:, (m * KH + c) * P:(m * KH + c + 1) * P]
                        nc.tensor.matmul(g[:, m:m + 1], blk, HB[:, ds(e * KH + c, 1)],
                                         start=False,
                                         stop=(m == MG - 1 and c == KH - 1))
                nc.scalar.activation(sg, g, AF.Sigmoid, scale=1.0 / XS)
                # c = f*c + i*(2*sg~-1) = f*c + 2*(i*sg~) - i
                nc.vector.tensor_mul(t2, sg[:, 4:8], cS)
                nc.vector.tensor_mul(t1, sg[:, 0:4], sg[:, 12:16])
                nc.vector.scalar_tensor_tensor(
                    tm, t1, 2.0, t2,
                    op0=mybir.AluOpType.mult, op1=mybir.AluOpType.add)
                nc.vector.scalar_tensor_tensor(
                    cS, sg[:, 0:4], -1.0, tm,
                    op0=mybir.AluOpType.mult, op1=mybir.AluOpType.add)
                # tanh(c) = 2*sigmoid(2c)-1 ; h = o*tanh(c)
                nc.scalar.activation(sc, cS, AF.Sigmoid, scale=2.0)
                nc.vector.scalar_tensor_tensor(
                    w4, sc, 2.0, ones4,
                    op0=mybir.AluOpType.mult, op1=mybir.AluOpType.subtract)
                # fp8 record (x HS) consumed by the recurrent matvec
                nc.vector.scalar_tensor_tensor(
                    HB[:, ds(e * KH + KH, KH)], sg[:, 8:12], HS, w4,
                    op0=mybir.AluOpType.mult, op1=mybir.AluOpType.mult)
                if ys:
                    nc.vector.tensor_mul(YS[:, ds(e * KH, KH)], sg[:, 8:12], w4)

            with tc.tile_pool(name="scan_ps", bufs=2, space="PSUM") as scan_ps, \
                 tc.tile_pool(name="x2_ps", bufs=2, space="PSUM") as x2_ps:
                with tc.For_i(0, NITER, 1,
                              hint_engines=(mybir.EngineType.PE,)) as i:
                    for u in range(U):
                        e = nc.snap(i * U + u)
                        chain_step(scan_ps, wa, X1T, H1, c1, e, "a")
                        chain_step(scan_ps, wb, X2T, H2, c2, e, "b", ys=True)
                    # X2 GEMM for next iteration's chain-B window
                    x2p = x2_ps.tile([P, MG, U], F32, tag="x2p")
                    for m in range(MG):
                        for c in range(KH):
                            blk = wi2[:, (m * KH + c) * P:(m * KH + c + 1) * P]
                            nc.tensor.matmul(x2p[:, m, :], blk,
                                             h1v[:, c, ds(i * U + 1, U)],
                                             start=(c == 0), stop=(c == KH - 1))
                    nc.vector.tensor_add(x2wv[:, :, ds(i * U + U, U)], x2p, b2rep)

            # --- phase D: vocab projection (ys rows start at slot U) ---
            ys_v = YS.rearrange("p (s c) -> p c s", c=KH)
            vts = [(o, min(512, VSLICE - o)) for o in range(0, VSLICE, 512)]
            gemm_ps = ctx.enter_context(tc.tile_pool(name="gemm_ps", bufs=2, space="PSUM"))
            for sb in range(L // P):
                for (voff, vlen) in vts:
                    ps = gemm_ps.tile([P, 512], F32, tag="gps")
                    for c in range(KH):
                        nc.tensor.matmul(ps[:, :vlen],
                                         ys_v[:, c, U + P * sb:U + P * (sb + 1)],
                                         woutt[:, c * VSLICE + voff:c * VSLICE + voff + vlen],
                                         start=(c == 0), stop=(c == KH - 1))
                    st = stage.tile([P, 512], F32, tag="gst")
                    nc.scalar.copy(st[:, :vlen], ps[:, :vlen])
                    nc.sync.dma_start(out=out_d[P * sb:P * (sb + 1), voff:voff + vlen],
                                      in_=st[:, :vlen])

    nc.compile()
    return nc


# ---------------------------------------------------------------------------
# public entry point
# ---------------------------------------------------------------------------

LAST_RESULT = None


def _in_maps(dev):
    maps = []
    for core in range(NCORES):
        maps.append(dict(
            xts=dev["xts"], w1i=dev["w1i"], b1s=dev["b1s"], wa=dev["wa"],
            wb=dev["wb"], wi2=dev["wi2"], b2rep=dev["b2rep"],
            ident=dev["ident"], woutt=dev["wouts"][core],
        ))
    return maps


def _cast_wdt(dev):
    """Cast the packed recurrent weights to the device dtype (ml_dtypes for
    fp8)."""
    if WDT_NAME == "fp8":
        import ml_dtypes
        dt = ml_dtypes.float8_e4m3
    else:
        dt = np.float16
    for k in ("wa", "wb", "wi2"):
        dev[k] = dev[k].astype(dt)
    return dev


def kernel(story_feature, captions, lengths, W_story, b_story, embed,
           W_ih1, W_hh1, b1, W_ih2, W_hh2, b2, W_out, b_out):
    global LAST_RESULT
    from concourse import bass_utils

    dev, meta = _host_pack(story_feature, captions, lengths, W_story, b_story,
                           embed, W_ih1, W_hh1, b1, W_ih2, W_hh2, b2, W_out, b_out)
    dev = _cast_wdt(dev)
    L = meta["L"]

    key = (L, U, WDT_NAME)
    if key not in _BUILD_CACHE:
        _BUILD_CACHE[key] = _build(L)
    nc = _BUILD_CACHE[key]

    trace = os.environ.get("BASS_TRACE", "0") == "1"
    res = bass_utils.run_bass_kernel_spmd(nc, _in_maps(dev),
                                          core_ids=list(range(NCORES)),
                                          trace=trace)
    LAST_RESULT = res

    logits = np.concatenate([res.results[c]["out"] for c in range(NCORES)],
                            axis=1)            # [L, V]
    return _host_post(logits, meta, b_out)


def _host_post(logits, meta, b_out):
    n_valid, bs, ts = meta["n_valid"], meta["bs"], meta["ts"]
    out = np.zeros((B, T, V), np.float32)
    out[:, 0, 1] = 10000.0
    rows = logits[:n_valid].astype(np.float32) + b_out.astype(np.float32)[None, :]
    out[bs, ts + 1] = rows
    return out


# revision 4
# speedup vs baseline: 1.0375x; 1.0133x over previous
"""Trainium2 Bass kernel for nn_DecoderStory_71880572666639 (V2).

Architecture: 2-layer LSTM (H=512) scanned sequentially over the compacted
valid (b,t) steps (hidden carries across the whole batch), followed by a
vocab projection V=10000.

V2 strategy (vs V1 baseline):
* Two independent sequential chains: layer-1 (needs only X1[t] + h1 rec)
  and layer-2 (needs X2[t] = W_ih2 @ h1[t] + b2 + h2 rec). Layer-2 runs C=U
  steps behind layer-1 on the SAME core; X2 is computed in chunked GEMMs
  (amortizing weight loads), not per-step matvecs. Per-step PE work drops
  from 192 to 128+eps weight tiles.
* Recurrent weights + hidden states in fp8e4 (x64 / x16 scale folding,
  un-scaled in the gate activation via ACT's scale arg) -> faster LDWEIGHTS.
* Gate tiles reordered (i,f,o | g~) so gate nonlinearity is 2 fused ACT
  instructions ([128,12] sigmoid + [128,4] tanh) instead of 32 per-column.
* The precomputed input projection X[t] is seeded into PSUM with a single
  identity matmul (start=True), so no per-column ACT bias is needed.
* All 8 cores run the identical scan; the vocab GEMM is split by vocab
  columns (1250 per core) exactly like V1.

Gate permutation: device gate j = 128*m + p (column m in [0,16), partition
p) maps to torch gate g = 512*T[m//4] + 128*(m%4) + p with T = (i,f,o,g~) =
(0,1,3,2); so columns 0-3 hold i, 4-7 f, 8-11 o, 12-15 g~, and hidden unit
u = 128*c + p lives at h[p, c] for c in [0,4).
"""

import os
import numpy as np

B, T, E, H, V = 64, 32, 256, 512, 10000
D1 = E + H            # 768
G = 4 * H             # 2048
P = 128
NCORES = 8
VSLICE = V // NCORES  # 1250
KC1 = D1 // P         # 6  K-chunks for the input projection
KH = H // P           # 4  K-chunks for one hidden vector
MG = G // P           # 16 gate tiles

U = int(os.environ.get("SCAN_UNROLL", "8"))     # unroll = chainB lag C
ABLATE = os.environ.get("SCAN_ABLATE", "0")     # "1": static h (timing only)
WDT_NAME = os.environ.get("SCAN_WDT", "fp8")    # "fp8" | "fp16"
WS = 64.0             # recurrent weight scale
HS = 16.0             # hidden state scale
XS = WS * HS          # 1024: scale of X1/X2/psum gate pre-activations
FLUSH = -60000.0      # forces sigmoid gates to 0 in chainB warmup steps

# device gate permutation (device j -> torch gate index); type order i,f,o,g
_m = np.arange(G) // P
_p = np.arange(G) % P
_TY = np.array([0, 1, 3, 2])  # i, f, o, g~
PERM = 512 * _TY[_m // 4] + P * (_m % 4) + _p      # [2048]


def _round_up(x, mult):
    return ((x + mult - 1) // mult) * mult


# ---------------------------------------------------------------------------
# host-side packing
# ---------------------------------------------------------------------------

def _pack_stationary(Wp: np.ndarray, kchunks: int) -> np.ndarray:
    """Pack a permuted weight matrix Wp [G, K] into the SBUF stationary
    layout [128, (MG*kchunks)*128], block order b = m*kchunks + k,
    block(m, k)[kk, mm] = Wp[128*m + mm, 128*k + kk]."""
    ksz = Wp.shape[1]
    assert ksz == kchunks * P
    v = Wp.reshape(MG, P, kchunks, P)           # [m, mm, k, kk]
    v = v.transpose(3, 0, 2, 1)                 # [kk, m, k, mm]
    return np.ascontiguousarray(v.reshape(P, MG * kchunks * P))


def _host_pack(story_feature, captions, lengths, W_story, b_story, embed,
               W_ih1, W_hh1, b1, W_ih2, W_hh2, b2, W_out, b_out):
    f32 = np.float32
    f16 = np.float16
    feats = np.maximum(story_feature.astype(f32) @ W_story.T.astype(f32)
                       + b_story.astype(f32), 0.0)          # [B, H]

    lengths = lengths.astype(np.int64)
    valid_pairs = [(b, t) for b in range(B) for t in range(int(lengths[b]) - 1)]
    n_valid = len(valid_pairs)
    L = max(_round_up(n_valid, max(P, U)), 2 * P)

    # x rows [feats; emb] in fp16, zero-padded to L+U steps
    x = np.zeros((L + U, D1), f16)
    bs = np.array([p[0] for p in valid_pairs])
    ts = np.array([p[1] for p in valid_pairs])
    x[:n_valid, :H] = feats[bs].astype(f16)
    x[:n_valid, H:] = embed[captions[bs, ts]].astype(f16)

    # xts: x.T chunked [128, KC1 * (L+U)]
    xT = np.ascontiguousarray(x.T)                        # [768, L+U]
    xts = xT.reshape(KC1, P, L + U).transpose(1, 0, 2).reshape(P, KC1 * (L + U))

    w1i = _pack_stationary(W_ih1[PERM].astype(f32) * XS, KC1).astype(f16)
    b1s = (b1[PERM].astype(f32) * XS).astype(f16).reshape(1, G)

    wa = _pack_stationary(W_hh1[PERM].astype(f32) * WS, KH)
    wb = _pack_stationary(W_hh2[PERM].astype(f32) * WS, KH)
    wi2 = _pack_stationary(W_ih2[PERM].astype(f32) * WS, KH)

    # b2 replicated over the GEMM window, layout [128, 16, U]
    b2t = np.ascontiguousarray(b2[PERM].reshape(MG, P).T).astype(f32) * XS
    b2rep = np.repeat(b2t[:, :, None], U, axis=2).astype(f16)  # [128,16,U]

    ident = np.eye(P, dtype=f16)

    # per-core W_out slices: woutt[kk, c*VSLICE + v] = W_out[v0+v, 128c+kk]
    wouts = []
    for core in range(NCORES):
        Woc = W_out[core * VSLICE:(core + 1) * VSLICE].astype(f32)   # [1250, 512]
        wt = Woc.T.reshape(KH, P, VSLICE).transpose(1, 0, 2).reshape(P, KH * VSLICE)
        wouts.append(np.ascontiguousarray(wt).astype(f16))

    meta = dict(n_valid=n_valid, L=L, bs=bs, ts=ts)
    dev = dict(
        xts=np.ascontiguousarray(xts).astype(f16),
        w1i=w1i, b1s=b1s, wa=wa, wb=wb, wi2=wi2,
        b2rep=np.ascontiguousarray(b2rep.reshape(P, MG * U)),
        ident=ident, wouts=wouts,
    )
    return dev, meta


# ---------------------------------------------------------------------------
# device kernel build
# ---------------------------------------------------------------------------

_BUILD_CACHE = {}


def _build(L):
    import concourse.bass as bass
    import concourse.tile as tile
    from concourse import bacc, mybir
    from concourse.bass import ds
    from contextlib import ExitStack

    F32 = mybir.dt.float32
    F16 = mybir.dt.float16
    WDT = mybir.dt.float8e4 if WDT_NAME == "fp8" else mybir.dt.float16
    AF = mybir.ActivationFunctionType
    NITER = L // U + 1
    SLOTS = L + U + 1          # h-state slots per chain

    nc = bacc.Bacc("TRN2", target_bir_lowering=False, debug=False,
                   num_devices=NCORES)

    xts_d = nc.dram_tensor("xts", [P, KC1 * (L + U)], F16, kind="ExternalInput").ap()
    w1i_d = nc.dram_tensor("w1i", [P, MG * KC1 * P], F16, kind="ExternalInput").ap()
    b1s_d = nc.dram_tensor("b1s", [1, G], F16, kind="ExternalInput").ap()
    wa_d = nc.dram_tensor("wa", [P, MG * KH * P], WDT, kind="ExternalInput").ap()
    wb_d = nc.dram_tensor("wb", [P, MG * KH * P], WDT, kind="ExternalInput").ap()
    wi2_d = nc.dram_tensor("wi2", [P, MG * KH * P], WDT, kind="ExternalInput").ap()
    b2rep_d = nc.dram_tensor("b2rep", [P, MG * U], F16, kind="ExternalInput").ap()
    ident_d = nc.dram_tensor("ident", [P, P], F16, kind="ExternalInput").ap()
    wout_d = nc.dram_tensor("woutt", [P, KH * VSLICE], F16, kind="ExternalInput").ap()
    out_d = nc.dram_tensor("out", [L, VSLICE], F32, kind="ExternalOutput").ap()

    with tile.TileContext(nc) as tc:
        with ExitStack() as ctx:
            singles = ctx.enter_context(tc.tile_pool(name="singles", bufs=1))
            stage = ctx.enter_context(tc.tile_pool(name="stage", bufs=2))

            # --- persistent SBUF tensors ---
            wa = singles.tile([P, MG * KH * P], WDT)
            wb = singles.tile([P, MG * KH * P], WDT)
            wi2 = singles.tile([P, MG * KH * P], WDT)
            w1i = singles.tile([P, MG * KC1 * P], F16)
            b1s = singles.tile([1, G], F16)
            b2rep = singles.tile([P, MG, U], F16)
            ident = singles.tile([P, P], F16)
            woutt = singles.tile([P, KH * VSLICE], F16)
            ones = singles.tile([1, 512], F16)
            nc.sync.dma_start(out=wa, in_=wa_d)
            nc.sync.dma_start(out=wb, in_=wb_d)
            nc.sync.dma_start(out=wi2, in_=wi2_d)
            nc.sync.dma_start(out=w1i, in_=w1i_d)
            nc.sync.dma_start(out=b1s, in_=b1s_d)
            nc.sync.dma_start(out=b2rep, in_=b2rep_d.rearrange("p (m u) -> p m u", u=U))
            nc.sync.dma_start(out=ident, in_=ident_d)
            nc.sync.dma_start(out=woutt, in_=wout_d)
            nc.vector.memset(ones, 1.0)

            X1T = singles.tile([P, 16 * (L + U)], F16)
            X2T = singles.tile([P, 16 * (L + 2 * U)], F16)
            H1 = singles.tile([P, 4 * SLOTS], WDT)
            H2 = singles.tile([P, 4 * SLOTS], WDT)
            YS = singles.tile([P, 4 * (L + U)], F16)

            # zero init / flush regions
            nc.vector.memset(H1[:, 0:4], 0.0)
            nc.vector.memset(H2[:, 0:4], 0.0)
            nc.vector.memset(X1T[:, 16 * L:], 0.0)
            x2v = X2T.rearrange("p (s m) -> p s m", m=16)
            nc.vector.memset(x2v[:, 0:U, 0:12], FLUSH)
            nc.vector.memset(x2v[:, 0:U, 12:16], 0.0)

            # --- phase B: input projection X1 = XS*(W_ih1 @ x + b1) ---
            xts_v = xts_d.rearrange("p (k l) -> p k l", k=KC1)
            x1tv = X1T.rearrange("p (s m) -> p m s", m=16)
            nts = [(o, min(512, L - o)) for o in range(0, L, 512)]
            with tc.tile_pool(name="xpool", bufs=2) as xpool, \
                 tc.tile_pool(name="pre_ps", bufs=4, space="PSUM") as pre_ps:
                for (off, nlen) in nts:
                    xsl = xpool.tile([P, KC1, 512], F16, tag="xsl")
                    nc.sync.dma_start(out=xsl[:, :, :nlen], in_=xts_v[:, :, off:off + nlen])
                    for m in range(MG):
                        ps = pre_ps.tile([P, 512], F32, tag="ps")
                        nc.tensor.matmul(ps[:, :nlen], b1s[0:1, P * m:P * (m + 1)],
                                         ones[0:1, :nlen], start=True, stop=False)
                        for kc in range(KC1):
                            blk = w1i[:, (m * KC1 + kc) * P:(m * KC1 + kc + 1) * P]
                            nc.tensor.matmul(ps[:, :nlen], blk, xsl[:, kc, :nlen],
                                             start=False, stop=(kc == KC1 - 1))
                        nc.vector.tensor_copy(x1tv[:, m, off:off + nlen], ps[:, :nlen])

            # --- the scan ---
            h1v = H1.rearrange("p (s c) -> p c s", c=KH)
            x2wv = X2T.rearrange("p (s m) -> p m s", m=16)

            c1 = singles.tile([P, KH], F32)
            c2 = singles.tile([P, KH], F32)
            nc.vector.memset(c1, 0.0)
            nc.vector.memset(c2, 0.0)

            def both_steps(scan_ps, e):
                # Two offset chains, stage-interleaved emission so neither
                # chain's ACT/DVE instructions queue behind the other's
                # blocked ones. Each ACT/DVE dependency link costs ~2us on
                # HW; the cell is the minimal 6-link form:
                # sigmoid/tanh(gates) -> i*g~ -> +f*c -> tanh(c) -> h8.
                ga = scan_ps.tile([P, MG], F32, tag="ga")
                gb = scan_ps.tile([P, MG], F32, tag="gb")
                sga = stage.tile([P, MG], F16, tag="sga")
                sgb = stage.tile([P, MG], F16, tag="sgb")
                tha = stage.tile([P, KH], F16, tag="tha")
                thb = stage.tile([P, KH], F16, tag="thb")
                t1a = stage.tile([P, KH], F32, tag="t1a")
                t2a = stage.tile([P, KH], F32, tag="t2a")
                t1b = stage.tile([P, KH], F32, tag="t1b")
                t2b = stage.tile([P, KH], F32, tag="t2b")
                # PE: chain A then chain B matvec groups
                for (g, W, XT, HB) in ((ga, wa, X1T, H1), (gb, wb, X2T, H2)):
                    nc.tensor.matmul(g[:, 0:MG], ident, XT[:, ds(e * MG, MG)],
                                     start=True, stop=False)
                    for m in range(MG):
                        for c in range(KH):
                            blk = W[:, (m * KH + c) * P:(m * KH + c + 1) * P]
                            mv = (HB[:, c:c + 1] if ABLATE == "1"
                                  else HB[:, ds(e * KH + c, 1)])
                            nc.tensor.matmul(g[:, m:m + 1], blk, mv,
                                             start=False,
                                             stop=(m == MG - 1 and c == KH - 1))
                # stage 1: gate activations (A then B on the ACT queue)
                nc.scalar.activation(sga[:, 0:12], ga[:, 0:12], AF.Sigmoid,
                                     scale=1.0 / XS)
                nc.scalar.activation(sga[:, 12:16], ga[:, 12:16], AF.Tanh,
                                     scale=1.0 / XS)
                nc.scalar.activation(sgb[:, 0:12], gb[:, 0:12], AF.Sigmoid,
                                     scale=1.0 / XS)
                nc.scalar.activation(sgb[:, 12:16], gb[:, 12:16], AF.Tanh,
                                     scale=1.0 / XS)
                # stage 2: c updates
                nc.vector.tensor_mul(t2a, sga[:, 4:8], c1)
                nc.vector.tensor_mul(t1a, sga[:, 0:4], sga[:, 12:16])
                nc.vector.tensor_add(c1, t1a, t2a)
                nc.vector.tensor_mul(t2b, sgb[:, 4:8], c2)
                nc.vector.tensor_mul(t1b, sgb[:, 0:4], sgb[:, 12:16])
                nc.vector.tensor_add(c2, t1b, t2b)
                # stage 3: tanh(c)
                nc.scalar.activation(tha, c1, AF.Tanh)
                nc.scalar.activation(thb, c2, AF.Tanh)
                # stage 4: h records (fp8 x HS) + ys
                nc.vector.scalar_tensor_tensor(
                    H1[:, ds(e * KH + KH, KH)], sga[:, 8:12], HS, tha,
                    op0=mybir.AluOpType.mult, op1=mybir.AluOpType.mult)
                nc.vector.scalar_tensor_tensor(
                    H2[:, ds(e * KH + KH, KH)], sgb[:, 8:12], HS, thb,
                    op0=mybir.AluOpType.mult, op1=mybir.AluOpType.mult)
                nc.vector.tensor_mul(YS[:, ds(e * KH, KH)], sgb[:, 8:12], thb)

            with tc.tile_pool(name="scan_ps", bufs=2, space="PSUM") as scan_ps, \
                 tc.tile_pool(name="x2_ps", bufs=2, space="PSUM") as x2_ps:
                with tc.For_i(0, NITER, 1,
                              hint_engines=(mybir.EngineType.PE,)) as i:
                    for u in range(U):
                        e = nc.snap(i * U + u)
                        both_steps(scan_ps, e)
                    # X2 GEMM for next iteration's chain-B window
                    x2p = x2_ps.tile([P, MG, U], F32, tag="x2p")
                    for m in range(MG):
                        for c in range(KH):
                            blk = wi2[:, (m * KH + c) * P:(m * KH + c + 1) * P]
                            nc.tensor.matmul(x2p[:, m, :], blk,
                                             h1v[:, c, ds(i * U + 1, U)],
                                             start=(c == 0), stop=(c == KH - 1))
                    nc.vector.tensor_add(x2wv[:, :, ds(i * U + U, U)], x2p, b2rep)

            # --- phase D: vocab projection (ys rows start at slot U) ---
            ys_v = YS.rearrange("p (s c) -> p c s", c=KH)
            vts = [(o, min(512, VSLICE - o)) for o in range(0, VSLICE, 512)]
            gemm_ps = ctx.enter_context(tc.tile_pool(name="gemm_ps", bufs=2, space="PSUM"))
            for sb in range(L // P):
                for (voff, vlen) in vts:
                    ps = gemm_ps.tile([P, 512], F32, tag="gps")
                    for c in range(KH):
                        nc.tensor.matmul(ps[:, :vlen],
                                         ys_v[:, c, U + P * sb:U + P * (sb + 1)],
                                         woutt[:, c * VSLICE + voff:c * VSLICE + voff + vlen],
                                         start=(c == 0), stop=(c == KH - 1))
                    st = stage.tile([P, 512], F32, tag="gst")
                    nc.scalar.copy(st[:, :vlen], ps[:, :vlen])
                    nc.sync.dma_start(out=out_d[P * sb:P * (sb + 1), voff:voff + vlen],
                                      in_=st[:, :vlen])

    nc.compile()
    return nc


# ---------------------------------------------------------------------------
# public entry point
# ---------------------------------------------------------------------------

LAST_RESULT = None


def _in_maps(dev):
    maps = []
    for core in range(NCORES):
        maps.append(dict(
            xts=dev["xts"], w1i=dev["w1i"], b1s=dev["b1s"], wa=dev["wa"],
            wb=dev["wb"], wi2=dev["wi2"], b2rep=dev["b2rep"],
            ident=dev["ident"], woutt=dev["wouts"][core],
        ))
    return maps


def _cast_wdt(dev):
    """Cast the packed recurrent weights to the device dtype (ml_dtypes for
    fp8)."""
    if WDT_NAME == "fp8":
        import ml_dtypes
        dt = ml_dtypes.float8_e4m3
    else:
        dt = np.float16
    for k in ("wa", "wb", "wi2"):
        dev[k] = dev[k].astype(dt)
    return dev


def kernel(story_feature, captions, lengths, W_story, b_story, embed,
           W_ih1, W_hh1, b1, W_ih2, W_hh2, b2, W_out, b_out):
    global LAST_RESULT
    from concourse import bass_utils

    dev, meta = _host_pack(story_feature, captions, lengths, W_story, b_story,
                           embed, W_ih1, W_hh1, b1, W_ih2, W_hh2, b2, W_out, b_out)
    dev = _cast_wdt(dev)
    L = meta["L"]

    key = (L, U, WDT_NAME)
    if key not in _BUILD_CACHE:
        _BUILD_CACHE[key] = _build(L)
    nc = _BUILD_CACHE[key]

    trace = os.environ.get("BASS_TRACE", "0") == "1"
    res = bass_utils.run_bass_kernel_spmd(nc, _in_maps(dev),
                                          core_ids=list(range(NCORES)),
                                          trace=trace)
    LAST_RESULT = res

    logits = np.concatenate([res.results[c]["out"] for c in range(NCORES)],
                            axis=1)            # [L, V]
    return _host_post(logits, meta, b_out)


def _host_post(logits, meta, b_out):
    n_valid, bs, ts = meta["n_valid"], meta["bs"], meta["ts"]
    out = np.zeros((B, T, V), np.float32)
    out[:, 0, 1] = 10000.0
    rows = logits[:n_valid].astype(np.float32) + b_out.astype(np.float32)[None, :]
    out[bs, ts + 1] = rows
    return out
